# revision 70
# baseline (speedup 1.0000x reference)
"""Causal attention (B=4, S=4096, D_in=768, D_out=64) on 8 trn2 NeuronCores.

Sharding: 2 cores per batch element. Core (b, p) handles query rows
{2*i + p} of batch b (row-interleaved => balanced causal work, identical
SPMD instruction stream). Host prep permutes x[b] rows to [own-parity,
other-parity], transposes to xT, and ships it as FP16 [128, 6, 4096]
(rel err vs the fp32 reference ~4e-4; tolerance 2e-2).

On-chip (fp16 operands everywhere, fp32 PSUM accumulation):
  Stage g (512-col block of xT), packed dual-purpose projections:
    g<4 (own parity):  pA[128,512] = [Wk | Wq*SCALE]^T @ blk -> kq_sb[g]
                         (partitions 0:64 = K^T, 64:128 = Q^T)
                       pV[128,512] = [Wq*SCALE | Wv]^T @ blk -> qv_sb[g]
                         (0:64 = Q^T again -- the scores own-row-group rhs
                          needs Q at partitions 0:64; packing it into the
                          V matmul makes that copy free -- 64:128 = V^T)
    g>=4 (other):      pA[128,512] = [Wv | Wk]^T @ blk -> kq_sb[g]
                         (0:64 = V^T, 64:128 = K^T)
  V^T halves are PE-transposed (identity at the matching partition range)
  to natural V tiles [128 keys, 65] whose col 64 is ones: the attn@V
  matmul then accumulates the softmax denominator for free.

  Scores for key-tile pair u = (tile u own, tile 16+u other):
    own  matmul: lhsT=K^T, rhs=Q^T both at partitions 0:64  -> rowgrp 0
    other matmul: both operands at partitions 64:128        -> rowgrp 64
  K=64 matmuls in distinct row-groups run CONCURRENTLY on the PE's
  16x(32x32) sub-arrays (~2x on hardware; the cost model serializes).
  at = exp(psS - 2) in fp16, one ACT op per pair ([128, 2, live] 3D AP);
  bias -2 cancels in softmax and keeps exp in fp16 range. Diagonal
  pairs restrict to the causally-live column range and get one DVE
  multiply by a {0,1} triangular mask (same mask for every T).
  attn@V: po[65, 512] += V_tile^T-stationary matmul over the live range
  (few large matmuls -- the PE weight-load path makes many small
  attn-stationary matmuls slower on hardware despite fewer streamed
  columns). Fully-masked sub-blocks are skipped everywhere.
  normalize: copy po to SBUF, PE-transpose per 128-q block, multiply by
  reciprocal of the denominator row, one batched DMA out.

Schedule (tuned against TimelineSim + repeat-diff HW timing): x blocks
prefetched 2 stages ahead on the SP queue (first blocks split for a
faster ramp); the exp table is preloaded at t=0 by a 1-element warmup
activation. Iteration t emits: proj(t) | T-1's two big diagonal pairs
(deferred from iteration t-1 -- their exps keep ACT fed across the
projection boundary) | T's non-diagonal pairs | normalize(t-1) |
proj(4+t) | T's two small diagonal pairs (their queued exps carry the
next boundary) | V transposes. attn@V is emitted four pairs late
(defer_attnv) with order-based PSUM start/stop flags, so score matmuls
never stall behind exp waits and diagonal attn@V only lands after its
V tiles exist. The last supertile runs all four diagonals big->small
so the final exp is tiny, and the epilogue interleaves the last attn@V
pops with per-128-column normalize chunks (chunk qs of po3 is final
once attnv(12+qs) ran -- later pairs only touch higher columns).
"""

from contextlib import ExitStack

import numpy as np

import concourse.mybir as mybir
import concourse.tile as tile
from concourse import bacc
from concourse.bass_utils import run_bass_kernel_spmd
from concourse.masks import make_identity

B, S, DI, DO = 4, 4096, 768, 64
NCORES = 8
SQ = S // 2          # 2048 local q rows per core
P = 128
NCH = DI // P        # 6 contraction chunks
NST = 4              # q supertiles per core
STW = 512            # supertile width
SCALE = 1.0 / np.sqrt(DO)
F16 = mybir.dt.float16
F32 = mybir.dt.float32
Exp = mybir.ActivationFunctionType.Exp
EXP_BIAS = -2.0
PROLOGUE = [(0, 3), (4, 2)]

_cache: dict = {}


def _emit_consts(nc, tc, pools, aps):
    """Weights/masks/identities loaded ONCE, outside the repeat body:
    they are invocation-invariant, and re-DMA-ing them per body makes
    body N+1's first weight DMA wait on body N's last weight reader,
    serializing successive invocations almost completely."""
    xt3, wkq, wvk2, tmask, out = aps
    consts = pools[0]
    # wkq first (gates the very first matmul), then the packed
    # [wqv|wkv] pair in one DMA; tmask rides the Pool queue
    wkq_sb = consts.tile([P, NCH * P], F16, tag="wkq")
    wvk2_sb = consts.tile([P, 2, NCH * P], F16, tag="wvk2")
    tmask_sb = consts.tile([P, 2, P], F16, tag="tmask")
    nc.gpsimd.dma_start(out=tmask_sb[:], in_=tmask[:])
    nc.sync.dma_start(out=wkq_sb[:], in_=wkq[:])
    nc.sync.dma_start(out=wvk2_sb[:], in_=wvk2[:])
    ident = consts.tile([DO + 1, DO + 1], F16, tag="ident")
    make_identity(nc, ident[:])
    identhi = consts.tile([P, DO], F16, tag="identhi")
    make_identity(nc, identhi[DO:P, 0:DO])
    identf = consts.tile([DO + 1, DO + 1], F32, tag="identf")
    make_identity(nc, identf[:])
    ebias = consts.tile([P, 1], F32, tag="ebias")
    nc.gpsimd.memset(ebias[:], EXP_BIAS)
    # warm up the exp table during the DMA ramp (LoadActFuncSet is ~1.3us;
    # without this it lands right before the first real exp)
    actwarm = consts.tile([P, 1], F32, tag="actwarm")
    nc.scalar.activation(actwarm[:], ebias[:], Exp, bias=ebias[:])
    return (wkq_sb, wvk2_sb, tmask_sb, ident, identhi, identf, ebias)


def _emit_body(nc, tc, pools, aps, C):
    xt3, wkq, wvk2, tmask, out = aps
    (consts, xpool, kqpool, vpool, attn_pool, osb,
     ps_proj, ps_s, ps_o) = pools
    wkq_sb, wvk2_sb, tmask_sb, ident, identhi, identf, ebias = C

    kq_sb = [None] * 8       # [128, 512]: g<4 lo=K^T hi=Q^T; g>=4 lo=V^T hi=K^T
    qv_of = [None] * NST     # [128, 512]: lo=Q^T dup, hi=V^T own
    qlo_sb = [None] * NST    # [64, 512] view of Q^T at partitions 0:64
    v_sb = [None] * (2 * 16) # [128, 65] natural V + ones col

    def kt_own(j):           # key tile j (0..15), partitions 0:64
        return kq_sb[j // 4][0:DO, (j % 4) * P:(j % 4 + 1) * P]

    def kt_oth(j):           # key tile 16+j, partitions 64:128
        return kq_sb[4 + j // 4][DO:P, (j % 4) * P:(j % 4 + 1) * P]

    xbs = [None] * 8

    def stage_dma(g, nsplit=1):
        xb = xpool.tile([P, NCH, STW], F16, tag="xblk", name=f"xb{g}")
        step = NCH // nsplit
        for c0 in range(0, NCH, step):
            nc.sync.dma_start(
                out=xb[:, c0:c0 + step, :],
                in_=xt3[:, c0:c0 + step, g * STW:(g + 1) * STW])
        xbs[g] = xb

    proj_ps = {}

    def proj_mm(g, which, c0, c1):
        """Chunks [c0, c1) of projection `which` ('a' or 'v') for stage g;
        allocates the PSUM tile at c0==0 and copies out after the last."""
        xb = xbs[g]
        key = (g, which)
        if c0 == 0:
            proj_ps[key] = ps_proj.tile([P, STW], F32, tag="proj",
                                        name=f"p{which.upper()}{g}")
        ps = proj_ps[key]
        for c in range(c0, c1):
            if which == "a":
                w = (wkq_sb[:, c * P:(c + 1) * P] if g < 4
                     else wvk2_sb[:, 1, c * P:(c + 1) * P])
            else:
                w = wvk2_sb[:, 0, c * P:(c + 1) * P]
            nc.tensor.matmul(ps[:], w, xb[:, c, :],
                             start=(c == 0), stop=(c == NCH - 1))
        if c1 == NCH:
            tag = f"kq{g}" if which == "a" else f"qv{g}"
            sb = kqpool.tile([P, STW], F16, tag=tag, name=tag)
            nc.vector.tensor_copy(sb[:], ps[:])
            if which == "a":
                kq_sb[g] = sb
            else:
                # [Wq | Wv]: lo 64 rows = Q^T at partitions 0:64 (the
                # scores own-row-group rhs), hi 64 rows = V^T.
                qlo_sb[g] = sb[0:DO, :]
                qv_of[g] = sb

    def stage_proj(g):
        proj_mm(g, "a", 0, NCH)
        if g < 4:
            proj_mm(g, "v", 0, NCH)

    def stage_vtr(g):
        # V-transposes deferred off the proj->scores critical window:
        # stage g's V tiles are first read by the diagonal pairs a phase
        # later, so emit them once the score/exp pipeline is refilled
        if g < 4:
            vsrc, voff, idv = qv_of[g], DO, identhi[DO:P, 0:DO]
        else:
            vsrc, voff, idv = kq_sb[g], 0, ident[0:DO, 0:DO]
        ptr = ps_proj.tile([P, 4, DO], F16, tag="proj", name=f"ptr{g}")
        for jj in range(4):
            nc.tensor.transpose(ptr[:, jj, :],
                                vsrc[voff:voff + DO, jj * P:(jj + 1) * P],
                                idv)
            j = 4 * g + jj if g < 4 else 16 + 4 * (g - 4) + jj
            v = vpool.tile([P, DO + 1], F16, tag=f"v{j}", name=f"v{j}")
            nc.vector.tensor_copy(v[:, 0:DO], ptr[:, jj, :])
            nc.gpsimd.memset(v[:, DO:DO + 1], 1.0)
            v_sb[j] = v

    po_tiles = [None] * NST
    pending = []  # software pipeline: emit attnV four exp-steps late
    attnv_done = [0] * NST  # pairs emitted per supertile (start/stop flags)

    def emit_attnv(T, u, h, at):
        m = u - 4 * T  # >= 0: diagonal pair
        c0 = P * m if m >= 0 else 0
        first = attnv_done[T] == 0
        attnv_done[T] += 1
        last = attnv_done[T] == 4 * T + 4
        for hh in ((0, 1) if h is None else (h,)):
            nc.tensor.matmul(
                po_tiles[T][:, c0:STW],
                v_sb[16 * hh + u][:],
                at[:, hh, c0:STW],
                start=(first and hh == 0),
                stop=(last and hh == 1))

    def defer_attnv(args):
        # depth 4: a full diagonal group stays pending until its V tiles
        # (transposed after the diag scores) exist
        while len(pending) >= 4:
            emit_attnv(*pending.pop(0))
        pending.append(args)

    def drain_attnv(n):
        for _ in range(min(n, len(pending))):
            emit_attnv(*pending.pop(0))

    def flush_attnv():
        while pending:
            emit_attnv(*pending.pop(0))

    def make_po(T):
        if po_tiles[T] is None:
            po_tiles[T] = ps_o.tile([DO + 1, STW], F32, tag="po",
                                    name=f"po{T}")

    def attention(T, us):
        make_po(T)
        for u in us:
            m = u - 4 * T
            c0 = P * m if m >= 0 else 0
            psS = ps_s.tile([P, 2, STW], F32, tag="s")
            nc.tensor.matmul(psS[:, 0, c0:STW], kt_own(u),
                             qlo_sb[T][:, c0:STW], start=True, stop=True)
            nc.tensor.matmul(psS[:, 1, c0:STW], kt_oth(u),
                             kq_sb[T][DO:P, c0:STW], start=True, stop=True)
            at = attn_pool.tile([P, 2, STW], F16, tag="at")
            # one ACT op: 3D AP [128, 2, live] covers both halves
            nc.scalar.activation(at[:, :, c0:STW], psS[:, :, c0:STW], Exp,
                                 bias=ebias[:])
            if m >= 0:  # triangular {0,1} mask on the diagonal blocks
                nc.vector.tensor_mul(at[:, :, c0:c0 + P], at[:, :, c0:c0 + P],
                                     tmask_sb[:])
            defer_attnv((T, u, None, at))

    def norm_chunk(T, qs, ot, ob):
        po = po_tiles[T]
        nc.vector.tensor_copy(ot[:, qs * P:(qs + 1) * P],
                              po[:, qs * P:(qs + 1) * P])
        ptr = ps_proj.tile([P, DO + 1], F32, tag="proj",
                           name=f"otr{T}_{qs}")
        nc.tensor.transpose(ptr[:], ot[:, qs * P:(qs + 1) * P],
                            identf[:])
        rc = osb.tile([P, 1], F32, tag="rc")
        nc.vector.reciprocal(rc[:], ptr[:, DO:DO + 1])
        nc.vector.tensor_scalar_mul(ob[:, qs, :], ptr[:, 0:DO], rc[:])

    def norm_out(T, ob):
        r0 = T * STW
        eng = nc.sync if T == NST - 1 else nc.gpsimd
        eng.dma_start(
            out=out[r0:r0 + STW, :].rearrange("(qs p) d -> p qs d", p=P),
            in_=ob[:])

    def normalize(T):
        ot = osb.tile([DO + 1, STW], F32, tag="ot")
        ob = osb.tile([P, 4, DO], F32, tag="ob")
        for qs in range(4):
            norm_chunk(T, qs, ot, ob)
        norm_out(T, ob)

    def start():
        # invocation prologue: x prefetch, the two T=0 projections, AND
        # T=0's small diagonal scores/exps. Emitted by the PREVIOUS
        # body's tail hook (software pipelining across repeat bodies):
        # the PE runs these under the previous body's diagonal exps and
        # this body's first exps queue right behind them on ACT, so ACT
        # barely gaps between invocations.
        for g, nsplit in PROLOGUE:
            stage_dma(g, nsplit=nsplit)
        stage_proj(0)
        stage_proj(4)
        attention(0, [2, 3])

    def rest(tail_hook=None):
        _loop(tail_hook)
        _epilogue()
        return None

    def _loop(tail_hook):
      for t in range(NST):
        if t > 0:
            stage_proj(t)
        if t > 0:
            # T-1's two big diagonal pairs (m=0,1) were held back: their
            # exps cover this iteration's projection/V-transpose stretch
            # so ACT never starves at the boundary
            attention(t - 1, [4 * (t - 1), 4 * (t - 1) + 1])
        if t > 0:
            # pop two pending attn@V first: PE filler while the first
            # nondiag scores wait this stage's PSUM->SBUF copies (at t=0
            # the pending entries' V tiles don't exist yet)
            drain_attnv(2)
        attention(t, list(range(4 * t)))       # non-diag: stages < t, 4..4+t-1
        if t > 0:
            # by 4+ pushes into this iteration the defer queue has fully
            # emitted T-1's attn@V
            normalize(t - 1)
        if t > 0:
            stage_proj(4 + t)
            # same filler trick as the iteration start: one ready attn@V
            # covers the diag scores' wait on kq(4+t)'s PSUM->SBUF copy
            drain_attnv(1)
        if 0 < t < 3:
            attention(t, [4 * t + 2, 4 * t + 3])  # small diagonals (m=2,3)
        elif t == 0:
            pass                                  # T=0 smalls ran in start()
        else:
            # tail: big->small so the last exp is tiny
            attention(3, [12, 13, 14, 15])
        stage_vtr(t)
        stage_vtr(4 + t)
        if t < 3:
            stage_dma(t + 1)
            stage_dma(4 + t + 1)
        elif tail_hook is not None:
            # next body's prologue lands here, under this body's
            # diagonal exps
            tail_hook()

    def _epilogue():
        # interleave the final attn@V pops with normalize(3) chunks;
        # chunk qs of po3 is final once attnv(12+qs) has run (later
        # diagonal pairs only touch columns >= 128*(qs+1))
        ot = osb.tile([DO + 1, STW], F32, tag="ot")
        ob = osb.tile([P, 4, DO], F32, tag="ob")
        for qs in range(4):
            drain_attnv(1)
            norm_chunk(NST - 1, qs, ot, ob)
        norm_out(NST - 1, ob)

    return start, rest


def _build_program(repeat: int = 1):
    """Build (and cache) the SPMD program. `repeat` re-emits the body N
    times in one NEFF (timing: the N-vs-1 diff cancels dispatch cost)."""
    if repeat in _cache:
        return _cache[repeat]
    nc = bacc.Bacc("TRN2", target_bir_lowering=False, debug=False)

    xt3 = nc.dram_tensor("xt3", [P, NCH, S], F16, kind="ExternalInput").ap()
    wkq = nc.dram_tensor("wkq", [P, NCH * P], F16, kind="ExternalInput").ap()
    wvk2 = nc.dram_tensor("wvk2", [P, 2, NCH * P], F16,
                          kind="ExternalInput").ap()
    tmask = nc.dram_tensor("tmask", [P, 2, P], F16, kind="ExternalInput").ap()
    out = nc.dram_tensor("out", [SQ, DO], F32, kind="ExternalOutput").ap()
    aps = (xt3, wkq, wvk2, tmask, out)

    with tile.TileContext(nc) as tc:
        with ExitStack() as ctx:
            pools = (
                ctx.enter_context(tc.tile_pool(name="consts", bufs=1)),
                ctx.enter_context(tc.tile_pool(name="xp", bufs=6)),
                # kq/v double-buffered so invocation N+1's projections
                # don't wait on invocation N's last attention reads
                ctx.enter_context(tc.tile_pool(name="kqp", bufs=2)),
                ctx.enter_context(tc.tile_pool(name="vp", bufs=2)),
                ctx.enter_context(tc.tile_pool(name="attn", bufs=16)),
                ctx.enter_context(tc.tile_pool(name="osb", bufs=4)),
                ctx.enter_context(tc.tile_pool(name="ps_proj", bufs=2,
                                               space="PSUM")),
                ctx.enter_context(tc.tile_pool(name="ps_s", bufs=2,
                                               space="PSUM")),
                ctx.enter_context(tc.tile_pool(name="ps_o", bufs=2,
                                               space="PSUM")),
            )
            C = _emit_consts(nc, tc, pools, aps)
            bodies = [_emit_body(nc, tc, pools, aps, C)
                      for _ in range(repeat)]
            bodies[0][0]()                       # first body's prologue
            for k in range(repeat):
                hook = bodies[k + 1][0] if k + 1 < repeat else None
                bodies[k][1](tail_hook=hook)     # body k + next prologue

    nc.compile()
    _cache[repeat] = nc
    return nc


def _perm(p: int) -> np.ndarray:
    return np.concatenate([np.arange(p, S, 2), np.arange(1 - p, S, 2)])


def make_in_maps(x, Wq, Wk, Wv):
    wq = np.asarray(Wq, np.float32) * np.float32(SCALE)
    wk = np.asarray(Wk, np.float32)
    wv = np.asarray(Wv, np.float32)

    def pack2(a, b):  # [768, 64] x2 -> [128, 6, 128] (lhsT chunks)
        m = np.concatenate([a, b], axis=1)          # [768, 128]
        return np.ascontiguousarray(
            m.reshape(NCH, P, P).transpose(1, 0, 2).astype(np.float16)
        ).reshape(P, NCH * P)

    wkq_h = pack2(wk, wq)                           # lo=K, hi=Q
    wkv_h = pack2(wv, wk)                           # lo=V, hi=K
    wqv_h = pack2(wq, wv)
    wvk2_h = np.ascontiguousarray(np.stack([wqv_h, wkv_h], axis=1))

    k = np.arange(P)[:, None]
    qi = np.arange(P)[None, :]
    masks = []
    for p in range(2):
        tm = np.empty((P, 2, P), np.float16)
        tm[:, 0, :] = (k <= qi)                     # own parity
        tm[:, 1, :] = (k <= qi + p - 1)             # other parity
        masks.append(tm)

    in_maps = []
    for c in range(NCORES):
        b, p = c // 2, c % 2
        xtc = np.asarray(x[b], np.float32)[_perm(p)].T.astype(np.float16)
        xt3 = np.ascontiguousarray(
            xtc.reshape(NCH, P, S).transpose(1, 0, 2))  # [128, 6, 4096]
        in_maps.append({
            "xt3": xt3, "wkq": wkq_h, "wvk2": wvk2_h,
            "tmask": masks[p],
        })
    return in_maps


def gather_out(results) -> np.ndarray:
    out = np.empty((B, S, DO), np.float32)
    for c in range(NCORES):
        b, p = c // 2, c % 2
        out[b, p::2, :] = results[c]["out"]
    return out


def run(x, Wq, Wk, Wv, trace=False, **spmd_kwargs):
    nc = _build_program()
    in_maps = make_in_maps(x, Wq, Wk, Wv)
    res = run_bass_kernel_spmd(
        nc, in_maps, core_ids=list(range(NCORES)), trace=trace, **spmd_kwargs)
    return gather_out(res.results), res


def kernel(x, Wq, Wk, Wv):
    out, _ = run(x, Wq, Wk, Wv)
    return out



# revision 71
# speedup vs baseline: 1.0946x; 1.0946x over previous
"""Causal attention (B=4, S=4096, D_in=768, D_out=64) on 8 trn2 NeuronCores.

Sharding: 2 cores per batch element. Core (b, p) handles query rows
{2*i + p} of batch b (row-interleaved => balanced causal work, identical
SPMD instruction stream). Host prep permutes x[b] rows to [own-parity,
other-parity], transposes to xT, and ships it as FP16 [128, 6, 4096]
(rel err vs the fp32 reference ~4e-4; tolerance 2e-2).

On-chip (fp16 operands everywhere, fp32 PSUM accumulation):
  Stage g (512-col block of xT), packed dual-purpose projections:
    g<4 (own parity):  pA[128,512] = [Wk | Wq*SCALE]^T @ blk -> kq_sb[g]
                         (partitions 0:64 = K^T, 64:128 = Q^T)
                       pV[128,512] = [Wq*SCALE | Wv]^T @ blk -> qv_sb[g]
                         (0:64 = Q^T again -- the scores own-row-group rhs
                          needs Q at partitions 0:64; packing it into the
                          V matmul makes that copy free -- 64:128 = V^T)
    g>=4 (other):      pA[128,512] = [Wv | Wk]^T @ blk -> kq_sb[g]
                         (0:64 = V^T, 64:128 = K^T)
  V^T halves are PE-transposed (identity at the matching partition range)
  to natural V tiles [128 keys, 65] whose col 64 is ones: the attn@V
  matmul then accumulates the softmax denominator for free.

  Scores for key-tile pair u = (tile u own, tile 16+u other):
    own  matmul: lhsT=K^T, rhs=Q^T both at partitions 0:64  -> rowgrp 0
    other matmul: both operands at partitions 64:128        -> rowgrp 64
  K=64 matmuls in distinct row-groups run CONCURRENTLY on the PE's
  16x(32x32) sub-arrays (~2x on hardware; the cost model serializes).
  at = exp(psS - 2) in fp16, one ACT op per pair ([128, 2, live] 3D AP);
  bias -2 cancels in softmax and keeps exp in fp16 range. Diagonal
  pairs restrict to the causally-live column range and get one DVE
  multiply by a {0,1} triangular mask (same mask for every T).
  attn@V: po[65, 512] += V_tile^T-stationary matmul over the live range
  (few large matmuls -- the PE weight-load path makes many small
  attn-stationary matmuls slower on hardware despite fewer streamed
  columns). Fully-masked sub-blocks are skipped everywhere.
  normalize: copy po to SBUF, PE-transpose per 128-q block, multiply by
  reciprocal of the denominator row, one batched DMA out.

Schedule (tuned against TimelineSim + repeat-diff HW timing): x blocks
prefetched 2 stages ahead on the SP queue (first blocks split for a
faster ramp); the exp table is preloaded at t=0 by a 1-element warmup
activation. Iteration t emits: proj(t) | T-1's two big diagonal pairs
(deferred from iteration t-1 -- their exps keep ACT fed across the
projection boundary) | T's non-diagonal pairs | normalize(t-1) |
proj(4+t) | T's two small diagonal pairs (their queued exps carry the
next boundary) | V transposes. attn@V is emitted four pairs late
(defer_attnv) with order-based PSUM start/stop flags, so score matmuls
never stall behind exp waits and diagonal attn@V only lands after its
V tiles exist. The last supertile runs all four diagonals big->small
so the final exp is tiny, and the epilogue interleaves the last attn@V
pops with per-128-column normalize chunks (chunk qs of po3 is final
once attnv(12+qs) ran -- later pairs only touch higher columns).
"""

from contextlib import ExitStack

import numpy as np

import concourse.mybir as mybir
import concourse.tile as tile
from concourse import bacc
from concourse.bass_utils import run_bass_kernel_spmd
from concourse.masks import make_identity

B, S, DI, DO = 4, 4096, 768, 64
NCORES = 8
SQ = S // 2          # 2048 local q rows per core
P = 128
NCH = DI // P        # 6 contraction chunks
NST = 4              # q supertiles per core
STW = 512            # supertile width
SCALE = 1.0 / np.sqrt(DO)
F16 = mybir.dt.float16
F32 = mybir.dt.float32
Exp = mybir.ActivationFunctionType.Exp
EXP_BIAS = -2.0
PROLOGUE = [(0, 3), (4, 2)]

_cache: dict = {}


def _emit_consts(nc, tc, pools, aps):
    """Weights/masks/identities loaded ONCE, outside the repeat body:
    they are invocation-invariant, and re-DMA-ing them per body makes
    body N+1's first weight DMA wait on body N's last weight reader,
    serializing successive invocations almost completely."""
    xt3, wkq, wvk2, tmask, out = aps
    consts = pools[0]
    # wkq first (gates the very first matmul), then the packed
    # [wqv|wkv] pair in one DMA; tmask rides the Pool queue
    wkq_sb = consts.tile([P, NCH * P], F16, tag="wkq")
    wvk2_sb = consts.tile([P, 2, NCH * P], F16, tag="wvk2")
    tmask_sb = consts.tile([P, 2, P], F16, tag="tmask")
    nc.gpsimd.dma_start(out=tmask_sb[:], in_=tmask[:])
    nc.sync.dma_start(out=wkq_sb[:], in_=wkq[:])
    nc.sync.dma_start(out=wvk2_sb[:], in_=wvk2[:])
    ident = consts.tile([DO + 1, DO + 1], F16, tag="ident")
    make_identity(nc, ident[:])
    identhi = consts.tile([P, DO], F16, tag="identhi")
    make_identity(nc, identhi[DO:P, 0:DO])
    identf = consts.tile([DO + 1, DO + 1], F32, tag="identf")
    make_identity(nc, identf[:])
    ebias = consts.tile([P, 1], F32, tag="ebias")
    nc.gpsimd.memset(ebias[:], EXP_BIAS)
    # warm up the exp table during the DMA ramp (LoadActFuncSet is ~1.3us;
    # without this it lands right before the first real exp)
    actwarm = consts.tile([P, 1], F32, tag="actwarm")
    nc.scalar.activation(actwarm[:], ebias[:], Exp, bias=ebias[:])
    return (wkq_sb, wvk2_sb, tmask_sb, ident, identhi, identf, ebias)


def _emit_body(nc, tc, pools, aps, C):
    xt3, wkq, wvk2, tmask, out = aps
    (consts, xpool, kqpool, vpool, attn_pool, osb,
     ps_proj, ps_s, ps_o) = pools
    wkq_sb, wvk2_sb, tmask_sb, ident, identhi, identf, ebias = C

    kq_sb = [None] * 8       # [128, 512]: g<4 lo=K^T hi=Q^T; g>=4 lo=V^T hi=K^T
    qv_of = [None] * NST     # [128, 512]: lo=Q^T dup, hi=V^T own
    qlo_sb = [None] * NST    # [64, 512] view of Q^T at partitions 0:64
    v_sb = [None] * (2 * 16) # [128, 65] natural V + ones col

    def kt_own(j):           # key tile j (0..15), partitions 0:64
        return kq_sb[j // 4][0:DO, (j % 4) * P:(j % 4 + 1) * P]

    def kt_oth(j):           # key tile 16+j, partitions 64:128
        return kq_sb[4 + j // 4][DO:P, (j % 4) * P:(j % 4 + 1) * P]

    xbs = [None] * 8

    def stage_dma(g, nsplit=1):
        xb = xpool.tile([P, NCH, STW], F16, tag="xblk", name=f"xb{g}")
        step = NCH // nsplit
        for c0 in range(0, NCH, step):
            nc.sync.dma_start(
                out=xb[:, c0:c0 + step, :],
                in_=xt3[:, c0:c0 + step, g * STW:(g + 1) * STW])
        xbs[g] = xb

    proj_ps = {}

    def proj_mm(g, which, c0, c1):
        """Chunks [c0, c1) of projection `which` ('a' or 'v') for stage g;
        allocates the PSUM tile at c0==0 and copies out after the last."""
        xb = xbs[g]
        key = (g, which)
        if c0 == 0:
            proj_ps[key] = ps_proj.tile([P, STW], F32, tag="proj",
                                        name=f"p{which.upper()}{g}")
        ps = proj_ps[key]
        for c in range(c0, c1):
            if which == "a":
                w = (wkq_sb[:, c * P:(c + 1) * P] if g < 4
                     else wvk2_sb[:, 1, c * P:(c + 1) * P])
            else:
                w = wvk2_sb[:, 0, c * P:(c + 1) * P]
            nc.tensor.matmul(ps[:], w, xb[:, c, :],
                             start=(c == 0), stop=(c == NCH - 1))
        if c1 == NCH:
            tag = f"kq{g}" if which == "a" else f"qv{g}"
            sb = kqpool.tile([P, STW], F16, tag=tag, name=tag)
            nc.vector.tensor_copy(sb[:], ps[:])
            if which == "a":
                kq_sb[g] = sb
            else:
                # [Wq | Wv]: lo 64 rows = Q^T at partitions 0:64 (the
                # scores own-row-group rhs), hi 64 rows = V^T.
                qlo_sb[g] = sb[0:DO, :]
                qv_of[g] = sb

    def stage_proj(g):
        proj_mm(g, "a", 0, NCH)
        if g < 4:
            proj_mm(g, "v", 0, NCH)

    def stage_vtr(g):
        # V-transposes deferred off the proj->scores critical window:
        # stage g's V tiles are first read by the diagonal pairs a phase
        # later, so emit them once the score/exp pipeline is refilled
        if g < 4:
            vsrc, voff, idv = qv_of[g], DO, identhi[DO:P, 0:DO]
        else:
            vsrc, voff, idv = kq_sb[g], 0, ident[0:DO, 0:DO]
        ptr = ps_proj.tile([P, 4, DO], F16, tag="proj", name=f"ptr{g}")
        for jj in range(4):
            nc.tensor.transpose(ptr[:, jj, :],
                                vsrc[voff:voff + DO, jj * P:(jj + 1) * P],
                                idv)
            j = 4 * g + jj if g < 4 else 16 + 4 * (g - 4) + jj
            v = vpool.tile([P, DO + 1], F16, tag=f"v{j}", name=f"v{j}")
            nc.vector.tensor_copy(v[:, 0:DO], ptr[:, jj, :])
            nc.gpsimd.memset(v[:, DO:DO + 1], 1.0)
            v_sb[j] = v

    po_tiles = [None] * NST
    pending = []  # software pipeline: emit attnV four exp-steps late
    attnv_done = [0] * NST  # pairs emitted per supertile (start/stop flags)

    def emit_attnv(T, u, h, at):
        m = u - 4 * T  # >= 0: diagonal pair
        c0 = P * m if m >= 0 else 0
        first = attnv_done[T] == 0
        attnv_done[T] += 1
        last = attnv_done[T] == 4 * T + 4
        for hh in ((0, 1) if h is None else (h,)):
            nc.tensor.matmul(
                po_tiles[T][:, c0:STW],
                v_sb[16 * hh + u][:],
                at[:, hh, c0:STW],
                start=(first and hh == 0),
                stop=(last and hh == 1))

    def defer_attnv(args):
        # depth 4: a full diagonal group stays pending until its V tiles
        # (transposed after the diag scores) exist
        while len(pending) >= 4:
            emit_attnv(*pending.pop(0))
        pending.append(args)

    def drain_attnv(n):
        for _ in range(min(n, len(pending))):
            emit_attnv(*pending.pop(0))

    def flush_attnv():
        while pending:
            emit_attnv(*pending.pop(0))

    def make_po(T):
        if po_tiles[T] is None:
            po_tiles[T] = ps_o.tile([DO + 1, STW], F32, tag="po",
                                    name=f"po{T}")

    def attention(T, us):
        make_po(T)
        for u in us:
            m = u - 4 * T
            c0 = P * m if m >= 0 else 0
            psS = ps_s.tile([P, 2, STW], F32, tag="s")
            nc.tensor.matmul(psS[:, 0, c0:STW], kt_own(u),
                             qlo_sb[T][:, c0:STW], start=True, stop=True)
            nc.tensor.matmul(psS[:, 1, c0:STW], kt_oth(u),
                             kq_sb[T][DO:P, c0:STW], start=True, stop=True)
            at = attn_pool.tile([P, 2, STW], F16, tag="at")
            # one ACT op: 3D AP [128, 2, live] covers both halves
            nc.scalar.activation(at[:, :, c0:STW], psS[:, :, c0:STW], Exp,
                                 bias=ebias[:])
            if m >= 0:  # triangular {0,1} mask on the diagonal blocks
                nc.vector.tensor_mul(at[:, :, c0:c0 + P], at[:, :, c0:c0 + P],
                                     tmask_sb[:])
            defer_attnv((T, u, None, at))

    def norm_chunk(T, qs, ot, ob):
        po = po_tiles[T]
        nc.vector.tensor_copy(ot[:, qs * P:(qs + 1) * P],
                              po[:, qs * P:(qs + 1) * P])
        ptr = ps_proj.tile([P, DO + 1], F32, tag="proj",
                           name=f"otr{T}_{qs}")
        nc.tensor.transpose(ptr[:], ot[:, qs * P:(qs + 1) * P],
                            identf[:])
        rc = osb.tile([P, 1], F32, tag="rc")
        nc.vector.reciprocal(rc[:], ptr[:, DO:DO + 1])
        nc.vector.tensor_scalar_mul(ob[:, qs, :], ptr[:, 0:DO], rc[:])

    def norm_out(T, ob):
        r0 = T * STW
        eng = nc.sync if T == NST - 1 else nc.gpsimd
        eng.dma_start(
            out=out[r0:r0 + STW, :].rearrange("(qs p) d -> p qs d", p=P),
            in_=ob[:])

    def normalize(T):
        ot = osb.tile([DO + 1, STW], F32, tag="ot")
        ob = osb.tile([P, 4, DO], F32, tag="ob")
        for qs in range(4):
            norm_chunk(T, qs, ot, ob)
        norm_out(T, ob)

    def start():
        # invocation prologue: x prefetch, the two T=0 projections, AND
        # T=0's small diagonal scores/exps. Emitted by the PREVIOUS
        # body's tail hook (software pipelining across repeat bodies):
        # the PE runs these under the previous body's diagonal exps and
        # this body's first exps queue right behind them on ACT, so ACT
        # barely gaps between invocations.
        for g, nsplit in PROLOGUE:
            stage_dma(g, nsplit=nsplit)
        stage_proj(0)
        stage_proj(4)
        attention(0, [2, 3])

    def rest(tail_hook=None):
        _loop(tail_hook)
        _epilogue()
        return None

    def _loop(tail_hook):
      for t in range(NST):
        if t > 0:
            stage_proj(t)
        if t > 0:
            # T-1's two big diagonal pairs (m=0,1) were held back: their
            # exps cover this iteration's projection/V-transpose stretch
            # so ACT never starves at the boundary
            attention(t - 1, [4 * (t - 1), 4 * (t - 1) + 1])
        if t > 0:
            # pop two pending attn@V first: PE filler while the first
            # nondiag scores wait this stage's PSUM->SBUF copies (at t=0
            # the pending entries' V tiles don't exist yet)
            drain_attnv(2)
        attention(t, list(range(4 * t)))       # non-diag: stages < t, 4..4+t-1
        if t > 0:
            # by 4+ pushes into this iteration the defer queue has fully
            # emitted T-1's attn@V
            normalize(t - 1)
        if t > 0:
            stage_proj(4 + t)
        if 0 < t < 3:
            attention(t, [4 * t + 2, 4 * t + 3])  # small diagonals (m=2,3)
        elif t == 0:
            pass                                  # T=0 smalls ran in start()
        else:
            # tail: big->small so the last exp is tiny
            attention(3, [12, 13, 14, 15])
        stage_vtr(t)
        stage_vtr(4 + t)
        if t < 3:
            stage_dma(t + 1)
            stage_dma(4 + t + 1)
        elif tail_hook is not None:
            # next body's prologue lands here, under this body's
            # diagonal exps
            tail_hook()

    def _epilogue():
        # interleave the final attn@V pops with normalize(3) chunks;
        # chunk qs of po3 is final once attnv(12+qs) has run (later
        # diagonal pairs only touch columns >= 128*(qs+1))
        ot = osb.tile([DO + 1, STW], F32, tag="ot")
        ob = osb.tile([P, 4, DO], F32, tag="ob")
        for qs in range(4):
            drain_attnv(1)
            norm_chunk(NST - 1, qs, ot, ob)
        norm_out(NST - 1, ob)

    return start, rest


def _build_program(repeat: int = 1):
    """Build (and cache) the SPMD program. `repeat` re-emits the body N
    times in one NEFF (timing: the N-vs-1 diff cancels dispatch cost)."""
    if repeat in _cache:
        return _cache[repeat]
    nc = bacc.Bacc("TRN2", target_bir_lowering=False, debug=False)

    xt3 = nc.dram_tensor("xt3", [P, NCH, S], F16, kind="ExternalInput").ap()
    wkq = nc.dram_tensor("wkq", [P, NCH * P], F16, kind="ExternalInput").ap()
    wvk2 = nc.dram_tensor("wvk2", [P, 2, NCH * P], F16,
                          kind="ExternalInput").ap()
    tmask = nc.dram_tensor("tmask", [P, 2, P], F16, kind="ExternalInput").ap()
    out = nc.dram_tensor("out", [SQ, DO], F32, kind="ExternalOutput").ap()
    aps = (xt3, wkq, wvk2, tmask, out)

    with tile.TileContext(nc) as tc:
        with ExitStack() as ctx:
            pools = (
                ctx.enter_context(tc.tile_pool(name="consts", bufs=1)),
                ctx.enter_context(tc.tile_pool(name="xp", bufs=6)),
                # kq/v double-buffered so invocation N+1's projections
                # don't wait on invocation N's last attention reads
                ctx.enter_context(tc.tile_pool(name="kqp", bufs=2)),
                ctx.enter_context(tc.tile_pool(name="vp", bufs=2)),
                ctx.enter_context(tc.tile_pool(name="attn", bufs=16)),
                ctx.enter_context(tc.tile_pool(name="osb", bufs=4)),
                ctx.enter_context(tc.tile_pool(name="ps_proj", bufs=2,
                                               space="PSUM")),
                ctx.enter_context(tc.tile_pool(name="ps_s", bufs=2,
                                               space="PSUM")),
                ctx.enter_context(tc.tile_pool(name="ps_o", bufs=2,
                                               space="PSUM")),
            )
            C = _emit_consts(nc, tc, pools, aps)
            bodies = [_emit_body(nc, tc, pools, aps, C)
                      for _ in range(repeat)]
            bodies[0][0]()                       # first body's prologue
            for k in range(repeat):
                hook = bodies[k + 1][0] if k + 1 < repeat else None
                bodies[k][1](tail_hook=hook)     # body k + next prologue

    nc.compile()
    _cache[repeat] = nc
    return nc


def _perm(p: int) -> np.ndarray:
    return np.concatenate([np.arange(p, S, 2), np.arange(1 - p, S, 2)])


def make_in_maps(x, Wq, Wk, Wv):
    wq = np.asarray(Wq, np.float32) * np.float32(SCALE)
    wk = np.asarray(Wk, np.float32)
    wv = np.asarray(Wv, np.float32)

    def pack2(a, b):  # [768, 64] x2 -> [128, 6, 128] (lhsT chunks)
        m = np.concatenate([a, b], axis=1)          # [768, 128]
        return np.ascontiguousarray(
            m.reshape(NCH, P, P).transpose(1, 0, 2).astype(np.float16)
        ).reshape(P, NCH * P)

    wkq_h = pack2(wk, wq)                           # lo=K, hi=Q
    wkv_h = pack2(wv, wk)                           # lo=V, hi=K
    wqv_h = pack2(wq, wv)
    wvk2_h = np.ascontiguousarray(np.stack([wqv_h, wkv_h], axis=1))

    k = np.arange(P)[:, None]
    qi = np.arange(P)[None, :]
    masks = []
    for p in range(2):
        tm = np.empty((P, 2, P), np.float16)
        tm[:, 0, :] = (k <= qi)                     # own parity
        tm[:, 1, :] = (k <= qi + p - 1)             # other parity
        masks.append(tm)

    in_maps = []
    for c in range(NCORES):
        b, p = c // 2, c % 2
        xtc = np.asarray(x[b], np.float32)[_perm(p)].T.astype(np.float16)
        xt3 = np.ascontiguousarray(
            xtc.reshape(NCH, P, S).transpose(1, 0, 2))  # [128, 6, 4096]
        in_maps.append({
            "xt3": xt3, "wkq": wkq_h, "wvk2": wvk2_h,
            "tmask": masks[p],
        })
    return in_maps


def gather_out(results) -> np.ndarray:
    out = np.empty((B, S, DO), np.float32)
    for c in range(NCORES):
        b, p = c // 2, c % 2
        out[b, p::2, :] = results[c]["out"]
    return out


def run(x, Wq, Wk, Wv, trace=False, **spmd_kwargs):
    nc = _build_program()
    in_maps = make_in_maps(x, Wq, Wk, Wv)
    res = run_bass_kernel_spmd(
        nc, in_maps, core_ids=list(range(NCORES)), trace=trace, **spmd_kwargs)
    return gather_out(res.results), res


def kernel(x, Wq, Wk, Wv):
    out, _ = run(x, Wq, Wk, Wv)
    return out

